# revision 3
# baseline (speedup 1.0000x reference)
"""MoE (top-2 of 8 experts + shared expert) Trainium2 Bass kernel.

Strategy (expert-parallel, host-prepped routing, bf16 compute):
  - Router (sigmoid gate + top-2) runs on the host in fp32; it produces the
    token->expert gather lists.
  - Core c computes expert c's SwiGLU FFN densely over the tokens routed to
    it (padded to the max per-expert count), plus the shared-expert FFN over
    the token shard [c*1024, (c+1)*1024).
  - All matmuls are bf16 (1 cycle/row PE rate, FWL weight loads hidden by
    the PE reorder window; rel err ~4e-3 end to end, well inside 2e-2).
  - The shared pass runs FIRST, streaming its weight slabs on the SP DMA
    ring, while the expert weights (17.3 MB bf16) preload into
    SBUF-resident tiles on the ACT DMA ring.  The expert pass then runs
    with zero weight traffic.
  - Host scatter-adds the per-expert outputs (scaled by combine weights)
    and the shared outputs into the final [8192, 2048] f32 result.

Everything on-device is feature-major ("K on partitions") so the x @ W.T
chains need no on-chip transposes:
  stage1:  h1T[m,:] = sum_k w1T[k, m].T @ xT[k, :]   (PSUM accum over k)
  g = silu(h1T) * h3T                                 (ACT + DVE, bf16 out)
  stage2:  yT[md,:] = sum_kh w2T[kh, md].T @ gT[kh,:]
"""

import os
import sys

for _p in ("/opt/trn_rl_repo", "/root/.axon_site/_ro/trn_rl_repo"):
    if os.path.isdir(_p) and _p not in sys.path:
        sys.path.insert(0, _p)

import numpy as np
import ml_dtypes

import concourse.bass as bass  # noqa: F401
import concourse.mybir as mybir
import concourse.tile as tile
from concourse import bacc
from concourse.bass_utils import run_bass_kernel_spmd

# Problem constants (hardcoded per spec)
N_TOK = 8192
D = 2048
H = 1408
E = 8
TOP_K = 2
ROUTE_SCALE = 1.0
P = 128
KD = D // P    # 16 k-tiles over D
MH = H // P    # 11 m-tiles over H
MD = D // P    # 16 m-tiles over D (stage 2 out)
SHARD = N_TOK // E  # 1024 shared-expert tokens per core

MAX_CHUNK = int(os.environ.get("MOE_MAX_CHUNK", "512"))

F32 = mybir.dt.float32
BF16 = mybir.dt.bfloat16
NP_BF16 = ml_dtypes.bfloat16
SILU = mybir.ActivationFunctionType.Silu

LAST_RESULTS = None  # BassKernelResults of the most recent run (for test.py)

SKIP_MM = bool(os.environ.get("MOE_SKIP_MM"))
SKIP_DMA = bool(os.environ.get("MOE_SKIP_DMA"))


def _chunks(T):
    """Split T (multiple of 128) into chunks of <=MAX_CHUNK, multiples of 128."""
    n = -(-T // MAX_CHUNK)
    sizes = []
    rem = T
    for i in range(n):
        left = n - i
        c = min(MAX_CHUNK, -(-rem // left + 127) // 128 * 128) if left > 1 else rem
        c = min(c, rem)
        sizes.append(c)
        rem -= c
    assert sum(sizes) == T and all(s % 128 == 0 for s in sizes), sizes
    return sizes


def _subs(Tc):
    """Split Tc into matmul free-dim slices of <=512."""
    out = []
    rem = Tc
    while rem > 512:
        take = 384 if rem == 640 else 512
        out.append(take)
        rem -= take
    if rem:
        out.append(rem)
    s0 = 0
    res = []
    for s in out:
        res.append((s0, s))
        s0 += s
    return res


def _emit_ffn(nc, pools, x_dram, y_dram, T, get_w13, get_w2):
    """Emit one feature-major SwiGLU FFN over T tokens.

    get_w13(m) -> (w1_ap, w3_ap) each [P, KD*P]; get_w2(md) -> [P, MH*P].
    """
    xpool, gpool, spool, ypool, psum = pools

    cs = 0
    for Tc in _chunks(T):
        x_tiles = []
        for k in range(KD):
            xt = xpool.tile([P, Tc], BF16, name=f"x{k}")
            SKIP_DMA or nc.sync.dma_start(xt[:], x_dram[k, :, cs:cs + Tc])
            x_tiles.append(xt)
        subs = _subs(Tc)
        g_tiles = []
        for m in range(MH):
            w1m, w3m = get_w13(m)
            gm = gpool.tile([P, Tc], BF16, name=f"g{m}")
            ps1 = [psum.tile([P, 512], F32, name="acc")[:, :sl] for _, sl in subs]
            ps3 = [psum.tile([P, 512], F32, name="acc")[:, :sl] for _, sl in subs]
            for k in range(KD):
                for j, (s0, sl) in enumerate(subs):
                    SKIP_MM or nc.tensor.matmul(
                        ps1[j], w1m[:, k * P:(k + 1) * P], x_tiles[k][:, s0:s0 + sl],
                        start=(k == 0), stop=(k == KD - 1),
                    )
                for j, (s0, sl) in enumerate(subs):
                    SKIP_MM or nc.tensor.matmul(
                        ps3[j], w3m[:, k * P:(k + 1) * P], x_tiles[k][:, s0:s0 + sl],
                        start=(k == 0), stop=(k == KD - 1),
                    )
            for j, (s0, sl) in enumerate(subs):
                st = spool.tile([P, 512], BF16, name="silu")[:, :sl]
                SKIP_MM or nc.scalar.activation(st, ps1[j], SILU)
                SKIP_MM or nc.vector.tensor_mul(gm[:, s0:s0 + sl], st, ps3[j])
            g_tiles.append(gm)
        for md in range(MD):
            w2m = get_w2(md)
            ym = ypool.tile([P, Tc], F32, name="ym")
            psy = [psum.tile([P, 512], F32, name="acc")[:, :sl] for _, sl in subs]
            for kh in range(MH):
                for j, (s0, sl) in enumerate(subs):
                    SKIP_MM or nc.tensor.matmul(
                        psy[j], w2m[:, kh * P:(kh + 1) * P], g_tiles[kh][:, s0:s0 + sl],
                        start=(kh == 0), stop=(kh == MH - 1),
                    )
            for j, (s0, sl) in enumerate(subs):
                SKIP_MM or nc.vector.tensor_copy(ym[:, s0:s0 + sl], psy[j])
            SKIP_MM or nc.sync.dma_start(y_dram[md, :, cs:cs + Tc], ym[:])
        cs += Tc


def _build_program(c_cap, loop_reps=1):
    nc = bacc.Bacc("TRN2", target_bir_lowering=False, debug=False, num_devices=E)
    xe = nc.dram_tensor("xe", [KD, P, c_cap], BF16, kind="ExternalInput").ap()
    xs = nc.dram_tensor("xs", [KD, P, SHARD], BF16, kind="ExternalInput").ap()
    # resident expert weights, partition-major (one big DMA each)
    w1r = nc.dram_tensor("w1r", [P, MH * KD * P], BF16, kind="ExternalInput").ap()
    w3r = nc.dram_tensor("w3r", [P, MH * KD * P], BF16, kind="ExternalInput").ap()
    w2r = nc.dram_tensor("w2r", [P, MD * MH * P], BF16, kind="ExternalInput").ap()
    # shared-expert weights, slab-major (streamed per m)
    sw1s = nc.dram_tensor("sw1s", [MH, P, KD * P], BF16, kind="ExternalInput").ap()
    sw3s = nc.dram_tensor("sw3s", [MH, P, KD * P], BF16, kind="ExternalInput").ap()
    sw2s = nc.dram_tensor("sw2s", [MD, P, MH * P], BF16, kind="ExternalInput").ap()
    ye = nc.dram_tensor("ye", [MD, P, c_cap], F32, kind="ExternalOutput").ap()
    ys = nc.dram_tensor("ys", [MD, P, SHARD], F32, kind="ExternalOutput").ap()

    with tile.TileContext(nc) as tc:
        with tc.tile_pool(name="res", bufs=1) as res, \
             tc.tile_pool(name="xpool", bufs=1) as xpool, \
             tc.tile_pool(name="wpool", bufs=2) as wpool, \
             tc.tile_pool(name="w2pool", bufs=3) as w2pool, \
             tc.tile_pool(name="gpool", bufs=1) as gpool, \
             tc.tile_pool(name="spool", bufs=2) as spool, \
             tc.tile_pool(name="ypool", bufs=2) as ypool, \
             tc.tile_pool(name="psum", bufs=8, space="PSUM") as psum:
            pools = (xpool, gpool, spool, ypool, psum)

            def body():
                # resident expert weights: preload on the ACT DMA ring so
                # the shared pass's SP-ring streaming is not queued behind it
                w1t = res.tile([P, MH * KD * P], BF16, name="w1t")
                w3t = res.tile([P, MH * KD * P], BF16, name="w3t")
                w2t = res.tile([P, MD * MH * P], BF16, name="w2t")
                if not SKIP_DMA:
                    nc.scalar.dma_start(w1t[:], w1r)
                    nc.scalar.dma_start(w3t[:], w3r)
                    nc.scalar.dma_start(w2t[:], w2r)

                def stream_w13(m):
                    w1m = wpool.tile([P, KD * P], BF16, name="w1m")
                    SKIP_DMA or nc.sync.dma_start(w1m[:], sw1s[m])
                    w3m = wpool.tile([P, KD * P], BF16, name="w3m")
                    SKIP_DMA or nc.sync.dma_start(w3m[:], sw3s[m])
                    return w1m[:], w3m[:]

                def stream_w2(md):
                    w2m = w2pool.tile([P, MH * P], BF16, name="w2m")
                    SKIP_DMA or nc.sync.dma_start(w2m[:], sw2s[md])
                    return w2m[:]

                def res_w13(m):
                    o = m * KD * P
                    return w1t[:, o:o + KD * P], w3t[:, o:o + KD * P]

                def res_w2(md):
                    o = md * MH * P
                    return w2t[:, o:o + MH * P]

                _emit_ffn(nc, pools, xs, ys, SHARD, stream_w13, stream_w2)
                _emit_ffn(nc, pools, xe, ye, c_cap, res_w13, res_w2)

            if loop_reps > 1:
                with tc.For_i(0, loop_reps, 1):
                    body()
            else:
                body()
    nc.compile()
    return nc


def _tile_w13_stream(w):
    # [H, D] -> [MH, P, KD*P] with slab[m, p, k*P+j] = w[m*P+j, k*P+p]
    return np.ascontiguousarray(
        w.reshape(MH, P, KD, P).transpose(0, 3, 2, 1).reshape(MH, P, KD * P)
    )


def _tile_w2_stream(w):
    # [D, H] -> [MD, P, MH*P] with slab[md, p, kh*P+j] = w[md*P+j, kh*P+p]
    return np.ascontiguousarray(
        w.reshape(MD, P, MH, P).transpose(0, 3, 2, 1).reshape(MD, P, MH * P)
    )


def _tile_w13_res(w):
    # [H, D] -> [P, MH*KD*P] with t[p, (m*KD+k)*P+j] = w[m*P+j, k*P+p]
    return np.ascontiguousarray(
        w.reshape(MH, P, KD, P).transpose(3, 0, 2, 1).reshape(P, MH * KD * P)
    )


def _tile_w2_res(w):
    # [D, H] -> [P, MD*MH*P] with t[p, (md*MH+kh)*P+j] = w[md*P+j, kh*P+p]
    return np.ascontiguousarray(
        w.reshape(MD, P, MH, P).transpose(3, 0, 2, 1).reshape(P, MD * MH * P)
    )


def _tile_x(xt):
    # [T, D] -> [KD, P, T]
    T = xt.shape[0]
    return np.ascontiguousarray(xt.reshape(T, KD, P).transpose(1, 2, 0))


def _untile_y(y):
    # [MD, P, T] -> [T, D]
    return y.transpose(2, 0, 1).reshape(y.shape[2], D)


def prepare(x, gate_w, expert_bias, w1, w2, w3, sw1, sw2, sw3):
    """Host routing + input prep. Returns (nc, in_maps, meta)."""
    x = np.ascontiguousarray(np.asarray(x, dtype=np.float32))
    gate_w = np.asarray(gate_w, dtype=np.float32)
    expert_bias = np.asarray(expert_bias, dtype=np.float32)
    w1 = np.asarray(w1, dtype=np.float32)
    w2 = np.asarray(w2, dtype=np.float32)
    w3 = np.asarray(w3, dtype=np.float32)
    sw1 = np.asarray(sw1, dtype=np.float32)
    sw2 = np.asarray(sw2, dtype=np.float32)
    sw3 = np.asarray(sw3, dtype=np.float32)

    # ---- host router (fp32, matches reference numerics) ----
    logits = x @ gate_w.T  # [N, E] f32
    scores = np.where(
        logits >= 0,
        1.0 / (1.0 + np.exp(-logits, dtype=np.float32)),
        np.exp(logits, dtype=np.float32) / (1.0 + np.exp(logits, dtype=np.float32)),
    ).astype(np.float32)
    biased = scores + expert_bias[None, :]
    i1 = np.argmax(biased, axis=1)
    tmp = biased.copy()
    tmp[np.arange(N_TOK), i1] = -np.inf
    i2 = np.argmax(tmp, axis=1)
    s1 = scores[np.arange(N_TOK), i1]
    s2 = scores[np.arange(N_TOK), i2]
    denom = s1 + s2 + np.float32(1e-20)
    c1 = (s1 / denom * np.float32(ROUTE_SCALE)).astype(np.float32)
    c2 = (s2 / denom * np.float32(ROUTE_SCALE)).astype(np.float32)

    idx_list, cw_list = [], []
    for e in range(E):
        m1 = i1 == e
        m2 = i2 == e
        idx = np.concatenate([np.nonzero(m1)[0], np.nonzero(m2)[0]])
        cw = np.concatenate([c1[m1], c2[m2]]).astype(np.float32)
        idx_list.append(idx)
        cw_list.append(cw)
    counts = [len(i) for i in idx_list]
    c_cap = max(512, -(-max(counts) // 128) * 128)

    # ---- build + compile the SPMD program for this capacity ----
    nc = _build_program(c_cap, loop_reps=int(os.environ.get("MOE_LOOP_REPS", "1")))

    # ---- per-core inputs ----
    x_bf = x.astype(NP_BF16)
    in_maps = []
    sw1s = _tile_w13_stream(sw1.astype(NP_BF16))
    sw3s = _tile_w13_stream(sw3.astype(NP_BF16))
    sw2s = _tile_w2_stream(sw2.astype(NP_BF16))
    for c in range(E):
        idx = idx_list[c]
        pad = c_cap - len(idx)
        idx_pad = np.concatenate([idx, np.zeros(pad, dtype=idx.dtype)]) if pad else idx
        in_maps.append({
            "xe": _tile_x(x_bf[idx_pad]),
            "xs": _tile_x(x_bf[c * SHARD:(c + 1) * SHARD]),
            "w1r": _tile_w13_res(w1[c].astype(NP_BF16)),
            "w3r": _tile_w13_res(w3[c].astype(NP_BF16)),
            "w2r": _tile_w2_res(w2[c].astype(NP_BF16)),
            "sw1s": sw1s,
            "sw3s": sw3s,
            "sw2s": sw2s,
        })

    meta = (idx_list, cw_list, counts)
    return nc, in_maps, meta


def combine(meta, results):
    """Scatter-add per-core outputs into the final [N, D] array."""
    idx_list, cw_list, counts = meta
    out = np.zeros((N_TOK, D), dtype=np.float32)
    for c in range(E):
        r = results[c]
        cnt = counts[c]
        if cnt:
            y_tok = _untile_y(r["ye"])[:cnt]
            out[idx_list[c]] += cw_list[c][:, None] * y_tok
        out[c * SHARD:(c + 1) * SHARD] += _untile_y(r["ys"])
    return out


def kernel(x, gate_w, expert_bias, w1, w2, w3, sw1, sw2, sw3):
    nc, in_maps, meta = prepare(x, gate_w, expert_bias, w1, w2, w3, sw1, sw2, sw3)
    global LAST_RESULTS
    res = run_bass_kernel_spmd(nc, in_maps, core_ids=list(range(E)))
    LAST_RESULTS = res
    return combine(meta, res.results)
